# revision 27
# baseline (speedup 1.0000x reference)
"""AFNO2D Bass kernel for 8 TRN2 NeuronCores.

Decomposition: the op is independent per (batch, 96-channel block) = 4*8 = 32
units; each core gets 4 units (zero communication). Per unit [H=128, W=128,
c=96]: rfft2 -> block complex MLP -> irfft2 -> +residual, all done as PE
matmuls with layouts chained so no transposes are needed:

  s1 W-DFT   lhsT=X_c[w,h] (per channel), rhs=[Cw|Sw'] packed  -> Y[h,(c,2m)]
  s2 H-DFT   lhsT=Y_m[h,c] (per m), rhs=[Ch|-Sh]/[Sh|Ch]      -> Z[c,(m,2k)]
  s3 MLP1    lhsT=W1-aug[97,96] (bias via ones-row), relu     -> O1[o,(m,2k)]
  s4 MLP2    lhsT=O1-aug[97,k] (per m), rhs=W2-aug[97,192]    -> O2[k,(m,2c)]
  s5 iH-DFT  lhsT=O2_c[k,m] (per c), split r/i psum tiles     -> V[mpack,(c,h)]
  s6 iW-DFT  lhsT=[A;B][mpack,w] K-packed + identity-matmul residual -> out
Host pre-transposes x to [u][w][c][h] so every DMA is contiguous; gpsimd
casting DMAs do fp32<->bf16 inline.
"""
import os
import numpy as np
import ml_dtypes

BF16 = ml_dtypes.bfloat16

H = 128
W = 128
WM = 65          # W//2+1
BS = 96          # block size (channels per block)
NU = 4           # units per core
NCORES = 8
NB = 8           # num blocks
LAMBD = 0.01
FD_UNIT = BS * H   # 12288 free elems per partition for [w,(c,h)] stores

LAST_EXEC_NS = None
_CACHE = {}


# ---------------------------------------------------------------- host math
def _dft_tables():
    w = np.arange(W)[:, None].astype(np.float64)
    m = np.arange(WM)[None, :].astype(np.float64)
    ang = 2 * np.pi * w * m / W
    Cw = np.cos(ang) / np.sqrt(W)
    Sw = -np.sin(ang) / np.sqrt(W)
    dftw = np.concatenate([Cw, Sw[:, 1:64]], axis=1)          # [128,128]

    h = np.arange(H)[:, None].astype(np.float64)
    k = np.arange(H)[None, :].astype(np.float64)
    ang2 = 2 * np.pi * h * k / H
    Ch = np.cos(ang2) / np.sqrt(H)
    Sh = np.sin(ang2) / np.sqrt(H)
    dfth_r = np.concatenate([Ch, -Sh], axis=1)                # [128,256]
    dfth_i = np.concatenate([Sh, Ch], axis=1)                 # [128,256]

    mm = np.arange(WM)[:, None].astype(np.float64)
    ww = np.arange(W)[None, :].astype(np.float64)
    ang3 = 2 * np.pi * mm * ww / W
    alpha = np.ones((WM, 1)); alpha[1:64] = 2.0
    # K-packed inverse-W matrix: rows 0..64 = A (cos terms, weights Vr_m),
    # rows 65..127 = B (sin terms, weights Vi_m for m=1..63)
    Aw = np.zeros((128, 128))
    Aw[0:WM] = alpha * np.cos(ang3) / np.sqrt(W)
    Aw[WM:128] = -(2.0 * np.sin(ang3[1:64])) / np.sqrt(W)
    Bw = np.zeros((128, 128))   # unused (kept for table layout stability)

    ident = np.eye(128)
    # packed: dftw | dfth_r | dfth_i | Ch | Sh | nSh | Ch2 | Aw | Bw | I
    # [Ch|Sh] at OFF_CH is s5's rhs-A; [nSh|Ch2] at OFF_NSH is s5's rhs-B
    packed = np.concatenate(
        [dftw, dfth_r, dfth_i, Ch, Sh, -Sh, Ch, Aw, Bw, ident], axis=1)
    return packed.astype(np.float32)


# column offsets in the packed dft table
OFF_DFTW = 0
OFF_DFTH_R = 128
OFF_DFTH_I = 384
OFF_CH = 640
OFF_SH = 768
OFF_NSH = 896
OFF_CH2 = 1024
OFF_DFTIA = 1152
OFF_DFTIB = 1280
OFF_IDENT = 1408
DFT_COLS = 1536


def _mlp_tables(w1c, b1c, w2c, b2c):
    """w1c:[2,4,96,96] b1c:[2,4,96] w2c:[2,4,96,96] b2c:[2,4,96] (4 local blocks).
    Returns mlp1 [97, 4*4*96], mlp2 [97, 4*2*192] fp32."""
    m1 = np.zeros((97, NU * 4 * 96), np.float32)
    m2 = np.zeros((97, NU * 2 * 192), np.float32)
    z96 = np.zeros((1, 96), np.float32)
    for u in range(NU):
        W1r, W1i = w1c[0, u], w1c[1, u]
        b1r, b1i = b1c[0, u], b1c[1, u]
        W2r, W2i = w2c[0, u], w2c[1, u]
        b2r, b2i = b2c[0, u], b2c[1, u]
        A1 = np.concatenate([W1r, b1r[None]], 0)
        B1 = np.concatenate([-W1i, z96], 0)
        C1 = np.concatenate([W1i, b1i[None]], 0)
        D1 = np.concatenate([W1r, z96], 0)
        for v, Mv in enumerate([A1, B1, C1, D1]):
            m1[:, (u * 4 + v) * 96:(u * 4 + v + 1) * 96] = Mv
        R1 = np.concatenate([np.concatenate([W2r, W2i], 1),
                             np.concatenate([b2r, b2i])[None]], 0)
        R2 = np.concatenate([np.concatenate([-W2i, W2r], 1),
                             np.zeros((1, 192), np.float32)], 0)
        m2[:, (u * 2 + 0) * 192:(u * 2 + 1) * 192] = R1
        m2[:, (u * 2 + 1) * 192:(u * 2 + 2) * 192] = R2
    return m1, m2


# ---------------------------------------------------------------- program
def _build_program(reps=1):
    from concourse import bass, bacc, mybir
    from concourse.tile import TileContext

    f32 = mybir.dt.float32
    bf16 = mybir.dt.bfloat16
    nc = bacc.Bacc()

    # all inputs pre-cast to bf16 on the host: halves the input DMA time
    # (the 6MB fp32 per-unit x reads were stalling the whole pipeline at
    # the unit0->unit1 boundary for ~18us).
    xin = nc.declare_dram_parameter("xin", [NU, 128, FD_UNIT], bf16, isOutput=False)
    dfts = nc.declare_dram_parameter("dfts", [128, DFT_COLS], bf16, isOutput=False)
    mlp1 = nc.declare_dram_parameter("mlp1", [97, NU * 4 * 96], bf16, isOutput=False)
    mlp2 = nc.declare_dram_parameter("mlp2", [97, NU * 2 * 192], bf16, isOutput=False)
    onesd = nc.declare_dram_parameter("ones", [1, 4096], bf16, isOutput=False)
    # bf16 out: xt is already bf16 so this loses nothing; host casts to f32.
    # DMA cost is charged on the (smaller) destination side.
    outp = nc.declare_dram_parameter("out", [NU, 128, FD_UNIT], bf16,
                                     isOutput=True)

    with TileContext(nc) as tc:
        with (
            tc.tile_pool(name="consts", bufs=1) as consts,
            tc.tile_pool(name="xpool", bufs=2) as xpool,
            tc.tile_pool(name="ypool", bufs=2) as ypool,
            tc.tile_pool(name="rings", bufs=1) as rings,
            tc.tile_pool(name="o2pool", bufs=1) as o2pool,
            tc.tile_pool(name="vpool", bufs=1) as vpool,
            tc.tile_pool(name="shr", bufs=2) as shr,
            tc.tile_pool(name="psum", bufs=4, space="PSUM") as psum,
        ):
            dftt = consts.tile([128, DFT_COLS], bf16)
            nc.gpsimd.dma_start(dftt[:], dfts[:])
            w1t = consts.tile([97, NU * 4 * 96], bf16)
            w2t = consts.tile([97, NU * 2 * 192], bf16)

            rhs_w = dftt[:, OFF_DFTW:OFF_DFTW + 128]
            rhs_hr = dftt[:, OFF_DFTH_R:OFF_DFTH_R + 256]
            rhs_hi = dftt[:, OFF_DFTH_I:OFF_DFTH_I + 256]
            rhs_5a = dftt[:, OFF_CH:OFF_CH + 256]      # [Ch|Sh]
            rhs_5b = dftt[:, OFF_NSH:OFF_NSH + 256]    # [-Sh|Ch]
            lhs_ia = dftt[:, OFF_DFTIA:OFF_DFTIA + 128]    # [A(65);B(63)]
            lhs_id = dftt[:, OFF_IDENT:OFF_IDENT + 128]

            # rings with ones-row preset (bias augmentation)
            zring = rings.tile([97, 4096], bf16)   # four 4m-slots of Z
            oring = rings.tile([97, 4096], bf16)   # four 4m-slots of O1
            neglam = consts.tile([128, 1], f32)
            zbias = consts.tile([128, 1], f32)

            def emit_consts():
                # emitted AFTER unit0's x DMA so s1 isn't queued behind them
                nc.gpsimd.dma_start(w1t[:], mlp1[:])
                nc.gpsimd.dma_start(w2t[:], mlp2[:])
                nc.gpsimd.dma_start(zring[96:97, :], onesd[:])
                nc.gpsimd.dma_start(oring[96:97, :], onesd[:])
                nc.gpsimd.memset(neglam[:], -LAMBD)
                nc.gpsimd.memset(zbias[:], 0.0)

            def emit_s1(u):
                """DMA unit u's input (chunked for u=0 so the PE can start
                early) and run the W-DFT. Returns (xt, yt).  x DMAs ride the
                SP hardware queue so the gpsimd queue (scatter copies) can't
                head-of-line block them."""
                xt = xpool.tile([128, FD_UNIT], bf16, tag="x")
                nchunk = 4 if u == 0 else 1
                cw = FD_UNIT // nchunk
                for ch in range(nchunk):
                    nc.gpsimd.dma_start(xt[:, ch * cw:(ch + 1) * cw],
                                        xin[u][:, ch * cw:(ch + 1) * cw])
                x3 = xt.rearrange("p (c h) -> p c h", c=BS)
                yt = ypool.tile([128, FD_UNIT], bf16, tag="y")
                for cg in range(12):            # 8 channels per psum tile
                    ps = psum.tile([128, 1024], f32, tag="ps")
                    for ci in range(8):
                        c = cg * 8 + ci
                        nc.tensor.matmul(ps[:, ci * 128:(ci + 1) * 128],
                                         x3[:, c, :], rhs_w,
                                         start=True, stop=True)
                    dst = yt[:, cg * 1024:(cg + 1) * 1024]
                    if cg % 2 == 1:
                        nc.vector.tensor_copy(dst, ps[:])
                    else:
                        nc.scalar.copy(dst, ps[:])
                return xt, yt

            def _unit_loop():
              # consts BEFORE unit0's x: w1t/w2t are small (~0.9MB) but gate
              # s3(g0); queued after the x chunks they landed at t=39us and
              # stalled the whole PE for ~8us.
              emit_consts()
              unit_s1 = {0: emit_s1(0)}
              s2_pre = {}
              s3_pre = {}
              for u in range(NU):
                xt, yt = unit_s1.pop(u)
                y3 = yt.rearrange("p (c mk) -> p mk c", c=BS)  # [h, mcomp, c]
                # o2t: [k, (c, mpack)] with col c*128+mp; mp=m -> O2r(m),
                # mp=64+m -> O2i(m) (m=1..63).  o2n: companion buffer with
                # mp=m -> -O2i(m) (m=0..64), mp=64+m -> O2r(m).  Together
                # they are the two 128-col stationary operands of s5's
                # full-partition iH-DFT per channel.
                o2t = o2pool.tile([128, BS * 128], bf16, tag="o2")
                o2n = o2pool.tile([128, BS * 128], bf16, tag="o2n")
                o2tv = o2t.rearrange("p (c mp) -> p c mp", c=BS)
                o2nv = o2n.rearrange("p (c mp) -> p c mp", c=BS)

                # ------- s2/s3/s4 over 4m groups, software-pipelined so the
                # PE always has the NEXT group's s2 runnable while this
                # group's drains complete.
                ngroups = (WM + 3) // 4         # 17, last group has 1 m

                def grp(g):
                    ms = list(range(g * 4, min(g * 4 + 4, WM)))
                    return ms, len(ms), (g % 4) * 1024

                def s2_parts(g, y3loc=None, dve=False):
                    """Per-m MM thunks + drain for the H-DFT.  s2's lhsT is
                    c-strided (LDW ~309ns vs 107ns stream) so its MMs are
                    interleaved with s3/s4 streams to hide the weight loads."""
                    y3g = y3 if y3loc is None else y3loc
                    ms, nm, slot = grp(g)
                    ps2 = psum.tile([128, 1024], f32, tag="ps")
                    thunks = []
                    for j, mi in enumerate(ms):
                        def t(j=j, mi=mi):
                            dst2 = ps2[0:BS, j * 256:(j + 1) * 256]
                            lr = y3g[:, mi, :]
                            if 1 <= mi <= 63:
                                nc.tensor.matmul(dst2, lr, rhs_hr,
                                                 start=True, stop=False)
                                li = y3g[:, 65 + (mi - 1), :]
                                nc.tensor.matmul(dst2, li, rhs_hi,
                                                 start=False, stop=True)
                            else:
                                nc.tensor.matmul(dst2, lr, rhs_hr,
                                                 start=True, stop=True)
                        thunks.append(t)

                    def drain():
                        zdst = zring[0:BS, slot:slot + nm * 256]
                        if dve:
                            nc.vector.tensor_copy(zdst, ps2[0:BS, 0:nm * 256])
                        else:
                            nc.scalar.copy(zdst, ps2[0:BS, 0:nm * 256])
                    return thunks, drain

                def emit_s2(g, y3loc=None, dve=False):
                    th, dr = s2_parts(g, y3loc, dve)
                    for t in th:
                        t()
                    dr()

                def s3_parts(g, uu=None):
                    uw = u if uu is None else uu
                    ms, nm, slot = grp(g)
                    zr = zring[:, slot:slot + nm * 256].rearrange(
                        "p (m tk) -> p m tk", m=nm)[:, :, 0:128]
                    zi = zring[:, slot:slot + nm * 256].rearrange(
                        "p (m tk) -> p m tk", m=nm)[:, :, 128:256]
                    A1 = w1t[:, (uw * 4 + 0) * 96:(uw * 4 + 1) * 96]
                    B1 = w1t[:, (uw * 4 + 1) * 96:(uw * 4 + 2) * 96]
                    C1 = w1t[:, (uw * 4 + 2) * 96:(uw * 4 + 3) * 96]
                    D1 = w1t[:, (uw * 4 + 3) * 96:(uw * 4 + 4) * 96]
                    ps3 = psum.tile([128, 1024], f32, tag="ps")
                    mm = nc.tensor.matmul
                    thunks = [
                        lambda: mm(ps3[0:BS, 0:nm * 128], A1, zr,
                                   start=True, stop=False),
                        lambda: mm(ps3[0:BS, 0:nm * 128], B1, zi,
                                   start=False, stop=True),
                        lambda: mm(ps3[0:BS, 512:512 + nm * 128], C1, zr,
                                   start=True, stop=False),
                        lambda: mm(ps3[0:BS, 512:512 + nm * 128], D1, zi,
                                   start=False, stop=True),
                    ]

                    def drain():
                        # single relu over both halves: src (i@512, m@128, k),
                        # dst (i@128, m@256, k) — same (i, m, k) iter order
                        odst = oring[0:BS, slot:slot + nm * 256].rearrange(
                            "p (m i k) -> p i m k", i=2, k=128)
                        src_ri = ps3[0:BS, :].rearrange(
                            "p (i m k) -> p i m k", i=2, k=128)[:, :, 0:nm, :]
                        nc.scalar.activation(odst, src_ri,
                                             mybir.ActivationFunctionType.Relu,
                                             bias=zbias[0:BS, :])
                    return thunks, drain

                def emit_s3(g, uu=None):
                    th, dr = s3_parts(g, uu)
                    for t in th:
                        t()
                    dr()

                def s4_parts(g):
                    ms, nm, slot = grp(g)
                    m0 = ms[0]
                    R1 = w2t[:, (u * 2 + 0) * 192:(u * 2 + 1) * 192]
                    R2 = w2t[:, (u * 2 + 1) * 192:(u * 2 + 2) * 192]
                    ps4 = psum.tile([128, 1024], f32, tag="ps")
                    thunks = []
                    for j, mi in enumerate(ms):
                        def t(j=j):
                            l1 = oring[:, slot + j * 256: slot + j * 256 + 128]
                            l2 = oring[:,
                                       slot + j * 256 + 128: slot + j * 256 + 256]
                            dst4 = ps4[:, j * 256:j * 256 + 192]
                            nc.tensor.matmul(dst4, l1, R1, start=True, stop=False)
                            nc.tensor.matmul(dst4, l2, R2, start=False, stop=True)
                        thunks.append(t)

                    def drain():
                        # softshrink as src - clip(src, +-lambda); the clip
                        # tile t serves every sign combination via operand
                        # order.
                        src192 = ps4.rearrange(
                            "p (m x) -> p m x", m=4)[:, 0:nm, 0:192]
                        t1 = shr.tile([128, 768], bf16, tag="t1")
                        t192 = t1.rearrange("p (m x) -> p m x", m=4)[:, 0:nm, :]
                        nc.vector.tensor_scalar(t192, src192, -LAMBD, LAMBD,
                                                mybir.AluOpType.max,
                                                mybir.AluOpType.min)
                        srcT = ps4.rearrange("p (m x) -> p x m", m=4)
                        tT = t1.rearrange("p (m x) -> p x m", m=4)
                        orS, oiS = srcT[:, 0:BS, 0:nm], srcT[:, BS:192, 0:nm]
                        orT, oiT = tT[:, 0:BS, 0:nm], tT[:, BS:192, 0:nm]
                        sub = mybir.AluOpType.subtract
                        # O2r -> o2t cols m ; -O2i -> o2n cols m
                        nc.vector.tensor_tensor(o2tv[:, :, m0:m0 + nm],
                                                orS, orT, sub)
                        nc.vector.tensor_tensor(o2nv[:, :, m0:m0 + nm],
                                                oiT, oiS, sub)
                        # mi columns (64+m) exist only for m=1..63
                        j0 = 1 if g == 0 else 0
                        if m0 < 64 and j0 < nm:
                            nc.vector.tensor_tensor(
                                o2tv[:, :, 64 + m0 + j0: 64 + m0 + nm],
                                oiS[:, :, j0:nm], oiT[:, :, j0:nm], sub)
                            nc.scalar.copy(
                                o2nv[:, :, 64 + m0 + j0: 64 + m0 + nm],
                                o2tv[:, :, m0 + j0: m0 + nm])
                    return thunks, drain

                def emit_s4(g):
                    th, dr = s4_parts(g)
                    for t in th:
                        t()
                    dr()

                pre = s2_pre.pop(u, 0)
                pre3 = s3_pre.pop(u, 0)
                for g in range(ngroups + 2):
                    if pre <= g < ngroups:
                        emit_s2(g, dve=(u == 0 and g < 4))
                    if 1 <= g < ngroups + 1 and g - 1 >= pre3:
                        emit_s3(g - 1)
                    if g >= 2:
                        emit_s4(g - 2)

                # prefetch + W-DFT of the next unit now: its matmuls fill
                # the PE bubbles at the s4->s5 and s5->s6 boundaries
                if u + 1 < NU:
                    unit_s1[u + 1] = emit_s1(u + 1)

                # ---------------- s5: iH-DFT, full-partition channel-packed.
                # Per channel: lhsT = o2t-col-block with Ch, accumulate
                # lhsT = o2n-col-block with Sh -> psum rows = mpack
                # (0..64 Vr, 65..127 Vi) directly in vt's layout.  Drain is a
                # straight [128,1024] psum->sbuf cast DMA on the free Pool
                # queue -- zero DVE/ACT cost.
                vt = vpool.tile([128, FD_UNIT], bf16, tag="v")
                rhs_c = dftt[:, OFF_CH:OFF_CH + 128]          # Ch [k,h]
                rhs_s = dftt[:, OFF_CH + 128:OFF_CH + 256]    # Sh [k,h]

                def emit_s5(cg):
                    ps5 = psum.tile([128, 1024], f32, tag="ps")
                    for ci in range(8):
                        c = cg * 8 + ci
                        d5 = ps5[:, ci * 128:(ci + 1) * 128]
                        nc.tensor.matmul(d5, o2t[:, c * 128:(c + 1) * 128],
                                         rhs_c, start=True, stop=False)
                        nc.tensor.matmul(d5, o2n[:, c * 128:(c + 1) * 128],
                                         rhs_s, start=False, stop=True)
                    # split the drain across ACT+DVE halves: the two run
                    # concurrently, halving the latency that gates s6
                    nc.scalar.copy(vt[:, cg * 1024:cg * 1024 + 512],
                                   ps5[:, 0:512])
                    nc.vector.tensor_copy(vt[:, cg * 1024 + 512:(cg + 1) * 1024],
                                          ps5[:, 512:1024])

                # s6: iW-DFT -> psum; residual added in the drain
                # (tensor_tensor psum+xt -> xt in place).  s6 chunk j
                # consumes exactly s5 tile j's vt span, so the two stages
                # are emitted interleaved.
                def emit_s6(j):
                    ps6 = psum.tile([128, 1024], f32, tag="ps")
                    cols = [(j * 2 + jj) * 512 for jj in range(2)]
                    tail = (u == NU - 1)   # last unit: PE+ACT idle at the end
                    for jj, col in enumerate(cols):
                        nc.tensor.matmul(ps6[:, jj * 512:(jj + 1) * 512],
                                         lhs_ia, vt[:, col:col + 512],
                                         start=True, stop=not tail)
                    dsto = xt[:, j * 1024:(j + 1) * 1024]
                    if tail:   # residual via identity matmul; drains split
                        for jj, col in enumerate(cols):
                            nc.tensor.matmul(ps6[:, jj * 512:(jj + 1) * 512],
                                             lhs_id, xt[:, col:col + 512],
                                             start=False, stop=True)
                        if j % 2 == 0:  # alternate ACT/DVE so neither backlogs
                            nc.scalar.copy(dsto, ps6[:])
                        else:
                            nc.vector.tensor_copy(dsto, ps6[:])
                    else:
                        nc.vector.tensor_tensor(dsto, ps6[:], dsto,
                                                mybir.AluOpType.add)
                    if j % 2 == 1:      # flush each 2048-col span as it lands
                        lo = (j - 1) * 1024
                        nc.gpsimd.dma_start(outp[u][:, lo:lo + 2048],
                                            xt[:, lo:lo + 2048])

                for cg in range(12):
                    emit_s5(cg)
                    if cg >= 1:
                        emit_s6(cg - 1)
                    # pre-emit next unit's first s2 groups into the tail: the
                    # rings are free and the drain engines have slack here
                    if 8 <= cg < 12 and u + 1 < NU:
                        y3n = unit_s1[u + 1][1].rearrange(
                            "p (c mk) -> p mk c", c=BS)
                        emit_s2(cg - 8, y3loc=y3n)
                        s2_pre[u + 1] = cg - 7
                emit_s6(11)

            import contextlib
            loop_cm = (tc.For_i(0, reps, 1) if reps > 1
                       else contextlib.nullcontext())
            with loop_cm:
                _unit_loop()

    nc.compile()
    return nc


# ---------------------------------------------------------------- entry
def _get_program(reps=1):
    key = f"nc{reps}"
    if key not in _CACHE:
        _CACHE[key] = _build_program(reps)
    return _CACHE[key]


def kernel(x, w1, b1, w2, b2, H=128, W=128):
    global LAST_EXEC_NS
    from concourse.bass_utils import run_bass_kernel_spmd

    x = np.asarray(x, np.float32)
    w1 = np.asarray(w1, np.float32); b1 = np.asarray(b1, np.float32)
    w2 = np.asarray(w2, np.float32); b2 = np.asarray(b2, np.float32)
    B, N, C = x.shape

    dft_packed = _dft_tables().astype(BF16)
    ones = np.ones((1, 4096), BF16)

    in_maps = []
    for core in range(NCORES):
        b = core // 2
        blk0 = 4 * (core % 2)
        # xin: [u, w, c, h] contiguous, pre-cast to bf16
        xb = x[b].reshape(128, 128, NB, BS)      # [h, w, blk, c]
        xu = np.ascontiguousarray(
            xb[:, :, blk0:blk0 + 4, :].transpose(2, 1, 3, 0)
        ).reshape(NU, 128, FD_UNIT).astype(BF16)  # [u, w, c*h]
        m1, m2 = _mlp_tables(w1[:, blk0:blk0 + 4], b1[:, blk0:blk0 + 4],
                             w2[:, blk0:blk0 + 4], b2[:, blk0:blk0 + 4])
        in_maps.append({"xin": xu, "dfts": dft_packed, "mlp1": m1.astype(BF16),
                        "mlp2": m2.astype(BF16), "ones": ones})

    nc = _get_program()
    trace = bool(int(os.environ.get("KTRACE", "0")))
    res = run_bass_kernel_spmd(nc, in_maps, core_ids=list(range(NCORES)),
                               trace=trace)
    LAST_EXEC_NS = res.exec_time_ns
    global LAST_RES
    LAST_RES = res

    out = np.zeros((B, N, C), np.float32)
    for core in range(NCORES):
        b = core // 2
        blk0 = 4 * (core % 2)
        o = res.results[core]["out"].reshape(NU, 128, BS, 128)  # [u,w,c,h]
        o = o.transpose(0, 3, 1, 2)               # [u, h, w, c]
        ob = out[b].reshape(128, 128, NB, BS)
        for u in range(NU):
            ob[:, :, blk0 + u, :] = o[u]
    return out



# revision 30
# speedup vs baseline: 1.0041x; 1.0041x over previous
"""AFNO2D Bass kernel for 8 TRN2 NeuronCores.

Decomposition: the op is independent per (batch, 96-channel block) = 4*8 = 32
units; each core gets 4 units (zero communication). Per unit [H=128, W=128,
c=96]: rfft2 -> block complex MLP -> irfft2 -> +residual, all done as PE
matmuls with layouts chained so no transposes are needed:

  s1 W-DFT   lhsT=X_c[w,h] (per channel), rhs=[Cw|Sw'] packed  -> Y[h,(c,2m)]
  s2 H-DFT   lhsT=Y_m[h,c] (per m), rhs=[Ch|-Sh]/[Sh|Ch]      -> Z[c,(m,2k)]
  s3 MLP1    lhsT=W1-aug[97,96] (bias via ones-row), relu     -> O1[o,(m,2k)]
  s4 MLP2    lhsT=O1-aug[97,k] (per m), rhs=W2-aug[97,192]    -> O2[k,(m,2c)]
  s5 iH-DFT  lhsT=O2_c[k,m] (per c), split r/i psum tiles     -> V[mpack,(c,h)]
  s6 iW-DFT  lhsT=[A;B][mpack,w] K-packed + identity-matmul residual -> out
Host pre-transposes x to [u][w][c][h] and pre-casts everything to bf16 so
every DMA is contiguous and half-width (fp32 x reads stalled the unit0->1
boundary ~18us).
"""
import os
import numpy as np
import ml_dtypes

BF16 = ml_dtypes.bfloat16

H = 128
W = 128
WM = 65          # W//2+1
BS = 96          # block size (channels per block)
NU = 4           # units per core
NCORES = 8
NB = 8           # num blocks
LAMBD = 0.01
FD_UNIT = BS * H   # 12288 free elems per partition for [w,(c,h)] stores

LAST_EXEC_NS = None
_CACHE = {}


# ---------------------------------------------------------------- host math
def _dft_tables():
    w = np.arange(W)[:, None].astype(np.float64)
    m = np.arange(WM)[None, :].astype(np.float64)
    ang = 2 * np.pi * w * m / W
    Cw = np.cos(ang) / np.sqrt(W)
    Sw = -np.sin(ang) / np.sqrt(W)
    dftw = np.concatenate([Cw, Sw[:, 1:64]], axis=1)          # [128,128]

    h = np.arange(H)[:, None].astype(np.float64)
    k = np.arange(H)[None, :].astype(np.float64)
    ang2 = 2 * np.pi * h * k / H
    Ch = np.cos(ang2) / np.sqrt(H)
    Sh = np.sin(ang2) / np.sqrt(H)
    dfth_r = np.concatenate([Ch, -Sh], axis=1)                # [128,256]
    dfth_i = np.concatenate([Sh, Ch], axis=1)                 # [128,256]

    mm = np.arange(WM)[:, None].astype(np.float64)
    ww = np.arange(W)[None, :].astype(np.float64)
    ang3 = 2 * np.pi * mm * ww / W
    alpha = np.ones((WM, 1)); alpha[1:64] = 2.0
    # K-packed inverse-W matrix: rows 0..64 = A (cos terms, weights Vr_m),
    # rows 65..127 = B (sin terms, weights Vi_m for m=1..63)
    Aw = np.zeros((128, 128))
    Aw[0:WM] = alpha * np.cos(ang3) / np.sqrt(W)
    Aw[WM:128] = -(2.0 * np.sin(ang3[1:64])) / np.sqrt(W)
    Bw = np.zeros((128, 128))   # unused (kept for table layout stability)

    ident = np.eye(128)
    # packed: dftw | dfth_r | dfth_i | Ch | Sh | nSh | Ch2 | Aw | Bw | I
    # [Ch|Sh] at OFF_CH is s5's rhs-A; [nSh|Ch2] at OFF_NSH is s5's rhs-B
    packed = np.concatenate(
        [dftw, dfth_r, dfth_i, Ch, Sh, -Sh, Ch, Aw, Bw, ident], axis=1)
    return packed.astype(np.float32)


# column offsets in the packed dft table
OFF_DFTW = 0
OFF_DFTH_R = 128
OFF_DFTH_I = 384
OFF_CH = 640
OFF_SH = 768
OFF_NSH = 896
OFF_CH2 = 1024
OFF_DFTIA = 1152
OFF_DFTIB = 1280
OFF_IDENT = 1408
DFT_COLS = 1536


def _mlp_tables(w1c, b1c, w2c, b2c):
    """w1c:[2,4,96,96] b1c:[2,4,96] w2c:[2,4,96,96] b2c:[2,4,96] (4 local blocks).
    Returns mlp1 [97, 4*4*96], mlp2 [97, 4*2*192] fp32."""
    m1 = np.zeros((97, NU * 4 * 96), np.float32)
    m2 = np.zeros((97, NU * 2 * 192), np.float32)
    z96 = np.zeros((1, 96), np.float32)
    for u in range(NU):
        W1r, W1i = w1c[0, u], w1c[1, u]
        b1r, b1i = b1c[0, u], b1c[1, u]
        W2r, W2i = w2c[0, u], w2c[1, u]
        b2r, b2i = b2c[0, u], b2c[1, u]
        A1 = np.concatenate([W1r, b1r[None]], 0)
        B1 = np.concatenate([-W1i, z96], 0)
        C1 = np.concatenate([W1i, b1i[None]], 0)
        D1 = np.concatenate([W1r, z96], 0)
        for v, Mv in enumerate([A1, B1, C1, D1]):
            m1[:, (u * 4 + v) * 96:(u * 4 + v + 1) * 96] = Mv
        R1 = np.concatenate([np.concatenate([W2r, W2i], 1),
                             np.concatenate([b2r, b2i])[None]], 0)
        R2 = np.concatenate([np.concatenate([-W2i, W2r], 1),
                             np.zeros((1, 192), np.float32)], 0)
        m2[:, (u * 2 + 0) * 192:(u * 2 + 1) * 192] = R1
        m2[:, (u * 2 + 1) * 192:(u * 2 + 2) * 192] = R2
    return m1, m2


# ---------------------------------------------------------------- program
def _build_program(reps=1):
    from concourse import bass, bacc, mybir
    from concourse.tile import TileContext

    f32 = mybir.dt.float32
    bf16 = mybir.dt.bfloat16
    nc = bacc.Bacc()

    # all inputs pre-cast to bf16 on the host: halves the input DMA time
    # (the 6MB fp32 per-unit x reads were stalling the whole pipeline at
    # the unit0->unit1 boundary for ~18us).
    xin = nc.declare_dram_parameter("xin", [NU, 128, FD_UNIT], bf16, isOutput=False)
    dfts = nc.declare_dram_parameter("dfts", [128, DFT_COLS], bf16, isOutput=False)
    mlp1 = nc.declare_dram_parameter("mlp1", [97, NU * 4 * 96], bf16, isOutput=False)
    mlp2 = nc.declare_dram_parameter("mlp2", [97, NU * 2 * 192], bf16, isOutput=False)
    onesd = nc.declare_dram_parameter("ones", [1, 4096], bf16, isOutput=False)
    # bf16 out: xt is already bf16 so this loses nothing; host casts to f32.
    # DMA cost is charged on the (smaller) destination side.
    outp = nc.declare_dram_parameter("out", [NU, 128, FD_UNIT], bf16,
                                     isOutput=True)

    with TileContext(nc) as tc:
        with (
            tc.tile_pool(name="consts", bufs=1) as consts,
            tc.tile_pool(name="xpool", bufs=2) as xpool,
            tc.tile_pool(name="ypool", bufs=2) as ypool,
            tc.tile_pool(name="rings", bufs=1) as rings,
            tc.tile_pool(name="o2pool", bufs=1) as o2pool,
            tc.tile_pool(name="vpool", bufs=1) as vpool,
            tc.tile_pool(name="shr", bufs=2) as shr,
            tc.tile_pool(name="psum", bufs=4, space="PSUM") as psum,
        ):
            dftt = consts.tile([128, DFT_COLS], bf16)
            nc.gpsimd.dma_start(dftt[:], dfts[:])
            w1t = consts.tile([97, NU * 4 * 96], bf16)
            w2t = consts.tile([97, NU * 2 * 192], bf16)

            rhs_w = dftt[:, OFF_DFTW:OFF_DFTW + 128]
            rhs_hr = dftt[:, OFF_DFTH_R:OFF_DFTH_R + 256]
            rhs_hi = dftt[:, OFF_DFTH_I:OFF_DFTH_I + 256]
            rhs_5a = dftt[:, OFF_CH:OFF_CH + 256]      # [Ch|Sh]
            rhs_5b = dftt[:, OFF_NSH:OFF_NSH + 256]    # [-Sh|Ch]
            lhs_ia = dftt[:, OFF_DFTIA:OFF_DFTIA + 128]    # [A(65);B(63)]
            lhs_id = dftt[:, OFF_IDENT:OFF_IDENT + 128]

            # rings with ones-row preset (bias augmentation)
            zring = rings.tile([97, 4096], bf16)   # four 4m-slots of Z
            oring = rings.tile([97, 4096], bf16)   # four 4m-slots of O1
            neglam = consts.tile([128, 1], f32)
            zbias = consts.tile([128, 1], f32)

            def emit_consts():
                # emitted AFTER unit0's x DMA so s1 isn't queued behind them
                nc.gpsimd.dma_start(w1t[:], mlp1[:])
                nc.gpsimd.dma_start(w2t[:], mlp2[:])
                nc.gpsimd.dma_start(zring[96:97, :], onesd[:])
                nc.gpsimd.dma_start(oring[96:97, :], onesd[:])
                nc.gpsimd.memset(neglam[:], -LAMBD)
                nc.gpsimd.memset(zbias[:], 0.0)

            def emit_s1(u):
                """DMA unit u's input (chunked for u=0 so the PE can start
                early) and run the W-DFT. Returns (xt, yt)."""
                xt = xpool.tile([128, FD_UNIT], bf16, tag="x")
                nchunk = 4 if u == 0 else 1
                cw = FD_UNIT // nchunk
                for ch in range(nchunk):
                    nc.gpsimd.dma_start(xt[:, ch * cw:(ch + 1) * cw],
                                        xin[u][:, ch * cw:(ch + 1) * cw])
                x3 = xt.rearrange("p (c h) -> p c h", c=BS)
                yt = ypool.tile([128, FD_UNIT], bf16, tag="y")
                for cg in range(12):            # 8 channels per psum tile
                    ps = psum.tile([128, 1024], f32, tag="ps")
                    for ci in range(8):
                        c = cg * 8 + ci
                        nc.tensor.matmul(ps[:, ci * 128:(ci + 1) * 128],
                                         x3[:, c, :], rhs_w,
                                         start=True, stop=True)
                    dst = yt[:, cg * 1024:(cg + 1) * 1024]
                    if cg % 2 == 1:
                        nc.vector.tensor_copy(dst, ps[:])
                    else:
                        nc.scalar.copy(dst, ps[:])
                return xt, yt

            def _unit_loop():
              # consts BEFORE unit0's x: w1t/w2t are small (~0.9MB) but gate
              # s3(g0); queued after the x chunks they landed at t=39us and
              # stalled the whole PE for ~8us.
              emit_consts()
              unit_s1 = {0: emit_s1(0)}
              s2_pre = {}
              s3_pre = {}
              for u in range(NU):
                xt, yt = unit_s1.pop(u)
                y3 = yt.rearrange("p (c mk) -> p mk c", c=BS)  # [h, mcomp, c]
                # o2t: [k, (c, mpack)] with col c*128+mp; mp=m -> O2r(m),
                # mp=64+m -> O2i(m) (m=1..63).  o2n: companion buffer with
                # mp=m -> -O2i(m) (m=0..64), mp=64+m -> O2r(m).  Together
                # they are the two 128-col stationary operands of s5's
                # full-partition iH-DFT per channel.
                o2t = o2pool.tile([128, BS * 128], bf16, tag="o2")
                o2n = o2pool.tile([128, BS * 128], bf16, tag="o2n")
                o2tv = o2t.rearrange("p (c mp) -> p c mp", c=BS)
                o2nv = o2n.rearrange("p (c mp) -> p c mp", c=BS)

                # ------- s2/s3/s4 over 4m groups, software-pipelined so the
                # PE always has the NEXT group's s2 runnable while this
                # group's drains complete.
                ngroups = (WM + 3) // 4         # 17, last group has 1 m

                def grp(g):
                    ms = list(range(g * 4, min(g * 4 + 4, WM)))
                    return ms, len(ms), (g % 4) * 1024

                def s2_parts(g, y3loc=None, dve=False):
                    """Per-m MM thunks + drain for the H-DFT.  s2's lhsT is
                    c-strided (LDW ~309ns vs 107ns stream) so its MMs are
                    interleaved with s3/s4 streams to hide the weight loads."""
                    y3g = y3 if y3loc is None else y3loc
                    ms, nm, slot = grp(g)
                    ps2 = psum.tile([128, 1024], f32, tag="ps")
                    thunks = []
                    for j, mi in enumerate(ms):
                        def t(j=j, mi=mi):
                            dst2 = ps2[0:BS, j * 256:(j + 1) * 256]
                            lr = y3g[:, mi, :]
                            if 1 <= mi <= 63:
                                nc.tensor.matmul(dst2, lr, rhs_hr,
                                                 start=True, stop=False)
                                li = y3g[:, 65 + (mi - 1), :]
                                nc.tensor.matmul(dst2, li, rhs_hi,
                                                 start=False, stop=True)
                            else:
                                nc.tensor.matmul(dst2, lr, rhs_hr,
                                                 start=True, stop=True)
                        thunks.append(t)

                    def drain():
                        zdst = zring[0:BS, slot:slot + nm * 256]
                        if dve:
                            nc.vector.tensor_copy(zdst, ps2[0:BS, 0:nm * 256])
                        else:
                            nc.scalar.copy(zdst, ps2[0:BS, 0:nm * 256])
                    return thunks, drain

                def emit_s2(g, y3loc=None, dve=False):
                    th, dr = s2_parts(g, y3loc, dve)
                    for t in th:
                        t()
                    dr()

                def s3_parts(g, uu=None):
                    uw = u if uu is None else uu
                    ms, nm, slot = grp(g)
                    zr = zring[:, slot:slot + nm * 256].rearrange(
                        "p (m tk) -> p m tk", m=nm)[:, :, 0:128]
                    zi = zring[:, slot:slot + nm * 256].rearrange(
                        "p (m tk) -> p m tk", m=nm)[:, :, 128:256]
                    A1 = w1t[:, (uw * 4 + 0) * 96:(uw * 4 + 1) * 96]
                    B1 = w1t[:, (uw * 4 + 1) * 96:(uw * 4 + 2) * 96]
                    C1 = w1t[:, (uw * 4 + 2) * 96:(uw * 4 + 3) * 96]
                    D1 = w1t[:, (uw * 4 + 3) * 96:(uw * 4 + 4) * 96]
                    ps3 = psum.tile([128, 1024], f32, tag="ps")
                    mm = nc.tensor.matmul
                    thunks = [
                        lambda: mm(ps3[0:BS, 0:nm * 128], A1, zr,
                                   start=True, stop=False),
                        lambda: mm(ps3[0:BS, 0:nm * 128], B1, zi,
                                   start=False, stop=True),
                        lambda: mm(ps3[0:BS, 512:512 + nm * 128], C1, zr,
                                   start=True, stop=False),
                        lambda: mm(ps3[0:BS, 512:512 + nm * 128], D1, zi,
                                   start=False, stop=True),
                    ]

                    def drain():
                        # single relu over both halves: src (i@512, m@128, k),
                        # dst (i@128, m@256, k) — same (i, m, k) iter order
                        odst = oring[0:BS, slot:slot + nm * 256].rearrange(
                            "p (m i k) -> p i m k", i=2, k=128)
                        src_ri = ps3[0:BS, :].rearrange(
                            "p (i m k) -> p i m k", i=2, k=128)[:, :, 0:nm, :]
                        nc.scalar.activation(odst, src_ri,
                                             mybir.ActivationFunctionType.Relu,
                                             bias=zbias[0:BS, :])
                    return thunks, drain

                def emit_s3(g, uu=None):
                    th, dr = s3_parts(g, uu)
                    for t in th:
                        t()
                    dr()

                def s4_parts(g):
                    ms, nm, slot = grp(g)
                    m0 = ms[0]
                    R1 = w2t[:, (u * 2 + 0) * 192:(u * 2 + 1) * 192]
                    R2 = w2t[:, (u * 2 + 1) * 192:(u * 2 + 2) * 192]
                    ps4 = psum.tile([128, 1024], f32, tag="ps")
                    thunks = []
                    for j, mi in enumerate(ms):
                        def t(j=j):
                            l1 = oring[:, slot + j * 256: slot + j * 256 + 128]
                            l2 = oring[:,
                                       slot + j * 256 + 128: slot + j * 256 + 256]
                            dst4 = ps4[:, j * 256:j * 256 + 192]
                            nc.tensor.matmul(dst4, l1, R1, start=True, stop=False)
                            nc.tensor.matmul(dst4, l2, R2, start=False, stop=True)
                        thunks.append(t)

                    def drain():
                        # softshrink as src - clip(src, +-lambda); the clip
                        # tile t serves every sign combination via operand
                        # order.
                        src192 = ps4.rearrange(
                            "p (m x) -> p m x", m=4)[:, 0:nm, 0:192]
                        t1 = shr.tile([128, 768], bf16, tag="t1")
                        t192 = t1.rearrange("p (m x) -> p m x", m=4)[:, 0:nm, :]
                        nc.vector.tensor_scalar(t192, src192, -LAMBD, LAMBD,
                                                mybir.AluOpType.max,
                                                mybir.AluOpType.min)
                        srcT = ps4.rearrange("p (m x) -> p x m", m=4)
                        tT = t1.rearrange("p (m x) -> p x m", m=4)
                        orS, oiS = srcT[:, 0:BS, 0:nm], srcT[:, BS:192, 0:nm]
                        orT, oiT = tT[:, 0:BS, 0:nm], tT[:, BS:192, 0:nm]
                        sub = mybir.AluOpType.subtract
                        # O2r -> o2t cols m ; -O2i -> o2n cols m
                        nc.vector.tensor_tensor(o2tv[:, :, m0:m0 + nm],
                                                orS, orT, sub)
                        nc.vector.tensor_tensor(o2nv[:, :, m0:m0 + nm],
                                                oiT, oiS, sub)
                        # mi columns (64+m) exist only for m=1..63
                        j0 = 1 if g == 0 else 0
                        if m0 < 64 and j0 < nm:
                            nc.vector.tensor_tensor(
                                o2tv[:, :, 64 + m0 + j0: 64 + m0 + nm],
                                oiS[:, :, j0:nm], oiT[:, :, j0:nm], sub)
                            nc.scalar.copy(
                                o2nv[:, :, 64 + m0 + j0: 64 + m0 + nm],
                                o2tv[:, :, m0 + j0: m0 + nm])
                    return thunks, drain

                def emit_s4(g):
                    th, dr = s4_parts(g)
                    for t in th:
                        t()
                    dr()

                pre = s2_pre.pop(u, 0)
                pre3 = s3_pre.pop(u, 0)
                for g in range(ngroups + 2):
                    if pre <= g < ngroups:
                        emit_s2(g, dve=(u == 0 and g < 4))
                    if 1 <= g < ngroups + 1 and g - 1 >= pre3:
                        emit_s3(g - 1)
                    if g >= 2:
                        emit_s4(g - 2)

                # prefetch + W-DFT of the next unit now: its matmuls fill
                # the PE bubbles at the s4->s5 and s5->s6 boundaries
                if u + 1 < NU:
                    unit_s1[u + 1] = emit_s1(u + 1)

                # ---------------- s5: iH-DFT, full-partition channel-packed.
                # Per channel: lhsT = o2t-col-block with Ch, accumulate
                # lhsT = o2n-col-block with Sh -> psum rows = mpack
                # (0..64 Vr, 65..127 Vi) directly in vt's layout.  Drain is a
                # straight [128,1024] psum->sbuf cast DMA on the free Pool
                # queue -- zero DVE/ACT cost.
                vt = vpool.tile([128, FD_UNIT], bf16, tag="v")
                rhs_c = dftt[:, OFF_CH:OFF_CH + 128]          # Ch [k,h]
                rhs_s = dftt[:, OFF_CH + 128:OFF_CH + 256]    # Sh [k,h]

                def emit_s5(cg):
                    ps5 = psum.tile([128, 1024], f32, tag="ps")
                    for ci in range(8):
                        c = cg * 8 + ci
                        d5 = ps5[:, ci * 128:(ci + 1) * 128]
                        nc.tensor.matmul(d5, o2t[:, c * 128:(c + 1) * 128],
                                         rhs_c, start=True, stop=False)
                        nc.tensor.matmul(d5, o2n[:, c * 128:(c + 1) * 128],
                                         rhs_s, start=False, stop=True)
                    vdst = vt[:, cg * 1024:(cg + 1) * 1024]
                    if cg % 2 == 0:
                        nc.scalar.copy(vdst, ps5[:])
                    else:
                        nc.vector.tensor_copy(vdst, ps5[:])

                # s6: iW-DFT -> psum; residual added in the drain
                # (tensor_tensor psum+xt -> xt in place).  s6 chunk j
                # consumes exactly s5 tile j's vt span, so the two stages
                # are emitted interleaved.
                def emit_s6(j):
                    ps6 = psum.tile([128, 1024], f32, tag="ps")
                    cols = [(j * 2 + jj) * 512 for jj in range(2)]
                    tail = (u == NU - 1)   # last unit: PE+ACT idle at the end
                    for jj, col in enumerate(cols):
                        nc.tensor.matmul(ps6[:, jj * 512:(jj + 1) * 512],
                                         lhs_ia, vt[:, col:col + 512],
                                         start=True, stop=not tail)
                    dsto = xt[:, j * 1024:(j + 1) * 1024]
                    if tail:   # residual via identity matmul; drains split
                        for jj, col in enumerate(cols):
                            nc.tensor.matmul(ps6[:, jj * 512:(jj + 1) * 512],
                                             lhs_id, xt[:, col:col + 512],
                                             start=False, stop=True)
                        if j % 2 == 0:  # alternate ACT/DVE so neither backlogs
                            nc.scalar.copy(dsto, ps6[:])
                        else:
                            nc.vector.tensor_copy(dsto, ps6[:])
                    else:
                        nc.vector.tensor_tensor(dsto, ps6[:], dsto,
                                                mybir.AluOpType.add)
                    if j % 2 == 1:      # flush each 2048-col span as it lands
                        lo = (j - 1) * 1024
                        nc.gpsimd.dma_start(outp[u][:, lo:lo + 2048],
                                            xt[:, lo:lo + 2048])

                for cg in range(12):
                    emit_s5(cg)
                    if cg >= 1:
                        emit_s6(cg - 1)
                    # pre-emit next unit's first s2 groups into the tail: the
                    # rings are free and the drain engines have slack here
                    if 8 <= cg < 12 and u + 1 < NU:
                        y3n = unit_s1[u + 1][1].rearrange(
                            "p (c mk) -> p mk c", c=BS)
                        emit_s2(cg - 8, y3loc=y3n)
                        s2_pre[u + 1] = cg - 7
                emit_s6(11)

            import contextlib
            loop_cm = (tc.For_i(0, reps, 1) if reps > 1
                       else contextlib.nullcontext())
            with loop_cm:
                _unit_loop()

    nc.compile()
    return nc


# ---------------------------------------------------------------- entry
def _get_program(reps=1):
    key = f"nc{reps}"
    if key not in _CACHE:
        _CACHE[key] = _build_program(reps)
    return _CACHE[key]


def kernel(x, w1, b1, w2, b2, H=128, W=128):
    global LAST_EXEC_NS
    from concourse.bass_utils import run_bass_kernel_spmd

    x = np.asarray(x, np.float32)
    w1 = np.asarray(w1, np.float32); b1 = np.asarray(b1, np.float32)
    w2 = np.asarray(w2, np.float32); b2 = np.asarray(b2, np.float32)
    B, N, C = x.shape

    dft_packed = _dft_tables().astype(BF16)
    ones = np.ones((1, 4096), BF16)

    in_maps = []
    for core in range(NCORES):
        b = core // 2
        blk0 = 4 * (core % 2)
        # xin: [u, w, c, h] contiguous, pre-cast to bf16
        xb = x[b].reshape(128, 128, NB, BS)      # [h, w, blk, c]
        xu = np.ascontiguousarray(
            xb[:, :, blk0:blk0 + 4, :].transpose(2, 1, 3, 0)
        ).reshape(NU, 128, FD_UNIT).astype(BF16)  # [u, w, c*h]
        m1, m2 = _mlp_tables(w1[:, blk0:blk0 + 4], b1[:, blk0:blk0 + 4],
                             w2[:, blk0:blk0 + 4], b2[:, blk0:blk0 + 4])
        in_maps.append({"xin": xu, "dfts": dft_packed, "mlp1": m1.astype(BF16),
                        "mlp2": m2.astype(BF16), "ones": ones})

    nc = _get_program()
    trace = bool(int(os.environ.get("KTRACE", "0")))
    res = run_bass_kernel_spmd(nc, in_maps, core_ids=list(range(NCORES)),
                               trace=trace)
    LAST_EXEC_NS = res.exec_time_ns
    global LAST_RES
    LAST_RES = res

    out = np.zeros((B, N, C), np.float32)
    for core in range(NCORES):
        b = core // 2
        blk0 = 4 * (core % 2)
        o = res.results[core]["out"].reshape(NU, 128, BS, 128)  # [u,w,c,h]
        o = o.transpose(0, 3, 1, 2)               # [u, h, w, c]
        ob = out[b].reshape(128, 128, NB, BS)
        for u in range(NU):
            ob[:, :, blk0 + u, :] = o[u]
    return out

